# revision 11
# baseline (speedup 1.0000x reference)
"""FP8 GEMM kernel for Trainium2 (8 NeuronCores, SPMD data-parallel over tokens).

Computes: out = fp16( fp32( e5m2(x) @ e4m3(weight.T) ) + bias )
  x      [4, 4096, 4096] fp16
  weight [4096, 4096]    fp16  (out_features, in_features)
  bias   [4096]          fp16
  out    [4, 4096, 4096] fp16

Sharding: token dim (B*S = 16384) split across 8 cores (2048 rows each);
weight + bias replicated. No collectives; host concatenates the outputs.

The host quantizes both operands to fp8 (ml_dtypes RNE — bit-identical to
the reference's own jnp casts) and pre-packs them into per-tile K-major
blocks (`[tile][ki=128][ko=32][free]`), so every device load is a plain
contiguous fp8 HWDGE DMA — no in-flight cast, half the bytes of an fp16
stream.  The bias is pre-broadcast on host to [128, 4096] so the device
load is one plain 1MB DMA instead of a slow replicating DMA.

Per-core kernel, v2 (~455us target vs the 442us fp8 DoubleRow floor =
2048 MMs x 215.8ns):
 - DoubleRow fp8 matmuls (K=256/instr, N=512) accumulate fp32 into PSUM;
   x (8MB fp8) stays resident, w n-tiles stream through a 3-deep pool.
 - ALL ramp-critical data (w0 + a combined x block for m=0..6) rides the
   sync queue EXCLUSIVELY, in consumption order.  (Trace evidence from
   v1: concurrent scalar-queue bulk loads starve the sync ramp stream
   3:1, delaying the first MM to 13.4us and stalling the warm-up.)
   Slack-tolerant loads (x7.., bias, w1..w7) queue on sync AFTER the
   ramp; stores ride scalar, so no load ever waits behind a store.
 - The PE clock starts gated at 1.2GHz and un-throttles after ~3.4us of
   sustained busy.  ~22 dummy N=128 matmuls on a zeroed scratch tile
   start the activity window at the preamble end (~7.2us), so the clock
   is at 2.4GHz by ~10.7us, right as the real stream takes over.
 - Warm-up interleaves m=0..6 k-chunk-outer across 7 PSUM banks (the
   8th holds the dummies), so each arriving k-chunk unlocks 7 matmuls:
   the PE consumes chunks slower than the (exclusive) sync queue
   delivers them — no mid-ramp stall, no clock re-throttle.
 - Bias add fused into the PSUM eviction on DVE (its only job).  The
   final group's eviction is split into 4x128-col strips stored
   alternately on scalar+sync to shorten the closing drain.
"""

import sys

if "/opt/trn_rl_repo" not in sys.path:
    sys.path.insert(0, "/opt/trn_rl_repo")

import ml_dtypes
import numpy as np

B, S, DIN, DOUT = 4, 4096, 4096, 4096
NCORES = 8
M_TOTAL = B * S              # 16384
M_LOC = M_TOTAL // NCORES    # 2048
P = 128
M_TILES = M_LOC // P         # 16 m-tiles of 128 rows
N_TILE = 512
N_TILES = DOUT // N_TILE     # 8
K_SUB = DIN // P             # 32 k-subtiles of 128
K_CHUNKS = K_SUB // 2        # 16 DoubleRow chunks of 256
WARM_M = 7                   # m-groups interleaved during the w0 ramp
N_DUMMY = 24                 # HAM-warming dummy matmuls (N=128)

_cached_nc = None


def _build():
    global _cached_nc
    if _cached_nc is not None:
        return _cached_nc

    import concourse.mybir as mybir
    import concourse.tile as tile
    from concourse import bacc

    nc = bacc.Bacc("TRN2", target_bir_lowering=False, debug=False,
                   num_devices=NCORES)

    # host-packed fp8 K-major tile blocks (see make_in_maps)
    xd01 = nc.dram_tensor("xd01", [P, WARM_M, K_SUB, P], mybir.dt.float8e5,
                          kind="ExternalInput")
    xd = nc.dram_tensor("xd", [M_TILES - WARM_M, P, K_SUB, P],
                        mybir.dt.float8e5, kind="ExternalInput")
    wd = nc.dram_tensor("wd", [N_TILES, P, K_SUB, N_TILE], mybir.dt.float8e4,
                        kind="ExternalInput")
    brep = nc.dram_tensor("brep", [P, DOUT], mybir.dt.float16,
                          kind="ExternalInput")
    out = nc.dram_tensor("out", [M_LOC, DOUT], mybir.dt.float16,
                         kind="ExternalOutput")

    with tile.TileContext(nc) as tc:
        with tc.tile_pool(name="w8p", bufs=3) as w8p, \
             tc.tile_pool(name="x8p", bufs=1) as x8p, \
             tc.tile_pool(name="outp", bufs=8) as outp, \
             tc.tile_pool(name="cst", bufs=1) as cst, \
             tc.tile_pool(name="psum", bufs=8, space="PSUM") as psump:

            # resident fp8 x: m=0..6 in one combined ko-major block (so one
            # ramp DMA delivers a k-slice for all warm groups at once), the
            # rest as per-m tiles
            x01 = x8p.tile([P, WARM_M, K_SUB, P], mybir.dt.float8e5,
                           tag="x01", name="x01")
            x8 = {m: x8p.tile([P, K_SUB, P], mybir.dt.float8e5,
                              tag=f"x8_{m}", name=f"x8_{m}")
                  for m in range(WARM_M, M_TILES)}

            def xap(m, kc):
                if m < WARM_M:
                    return x01[:, m, 2 * kc:2 * kc + 2, :]
                return x8[m][:, 2 * kc:2 * kc + 2, :]

            w8 = {}

            def load_w(j):
                w8[j] = w8p.tile([P, K_SUB, N_TILE], mybir.dt.float8e4,
                                 tag="w8", name=f"w8_{j}")
                nc.sync.dma_start(w8[j][:], wd[j, :, :, :])

            # ---- sync-queue program: strict priority order.  Emission
            # order = per-queue FIFO order; the HWDGE ring pops descriptors
            # in order, so data ARRIVES in consumption order.  Nothing else
            # rides sync until the tail, so the ramp gets the full ~350GB/s.
            # Steps are ONE k-chunk each: a chunk's matmuls gate on the
            # completion semaphore of the transfer that wrote it, so any
            # multi-chunk transfer stalls the PE until its LAST byte (v2/v3
            # measured ~3us stalls on every coarse step).  kc0 is split
            # even finer so the very first matmul unblocks on 96KB.
            w8[0] = w8p.tile([P, K_SUB, N_TILE], mybir.dt.float8e4,
                             tag="w8", name="w8_0")
            nc.sync.dma_start(w8[0][:, 0:2, :], wd[0, :, 0:2, :])
            nc.sync.dma_start(x01[:, 0:1, 0:2, :], xd01[:, 0:1, 0:2, :])
            nc.sync.dma_start(x01[:, 1:WARM_M, 0:2, :],
                              xd01[:, 1:WARM_M, 0:2, :])
            for k in range(1, K_CHUNKS):
                a, b = 2 * k, 2 * k + 2
                nc.sync.dma_start(w8[0][:, a:b, :], wd[0, :, a:b, :])
                nc.sync.dma_start(x01[:, :, a:b, :], xd01[:, :, a:b, :])
            bias_rep = cst.tile([P, DOUT], mybir.dt.float16)
            nc.sync.dma_start(bias_rep[:], brep.ap())
            for m in range(WARM_M, M_TILES):
                nc.sync.dma_start(x8[m][:], xd[m - WARM_M, :, :, :])
            for j in range(1, N_TILES):
                # w3.. pushes wait on the 3-deep pool's anti-dep (column
                # j-3 finished) — head-of-line blocking is fine, nothing
                # urgent behind them on sync
                load_w(j)

            # ---- HAM warm-up: dummy matmuls on a zeroed scratch tile keep
            # the PE activity window busy from the preamble end until the
            # first real chunk lands (~9.5us), so the 2.4GHz un-throttle
            # fires at ~10.7us instead of ~16.5us.
            dum = cst.tile([P, 2, P], mybir.dt.float8e5, name="dum")
            nc.gpsimd.memset(dum[:], 0)
            psum = {}
            dps = psump.tile([P, P], mybir.dt.float32, tag="ps", name="ps_dum")

            def dummies(n):
                for _ in range(n):
                    nc.tensor.matmul(
                        dps[:], dum[:], dum[:], start=True, stop=True,
                        perf_mode=mybir.MatmulPerfMode.DoubleRow,
                    )

            dummies(N_DUMMY)

            def mm(j, m, kc):
                nc.tensor.matmul(
                    psum[m][:],
                    xap(m, kc),
                    w8[j][:, 2 * kc:2 * kc + 2, :],
                    start=(kc == 0),
                    stop=(kc == K_CHUNKS - 1),
                    perf_mode=mybir.MatmulPerfMode.DoubleRow,
                )

            def evict(j, m, split=False):
                if not split:
                    ob = outp.tile([P, N_TILE], mybir.dt.float16, tag="ob",
                                   name=f"ob_{j}_{m}")
                    nc.vector.tensor_add(
                        ob[:], psum[m][:],
                        bias_rep[:, j * N_TILE:(j + 1) * N_TILE])
                    nc.scalar.dma_start(
                        out[m * P:(m + 1) * P,
                            j * N_TILE:(j + 1) * N_TILE], ob[:])
                    return
                # final group: halve the eviction and alternate the stores
                # across both HWDGE queues so the closing HBM-write receipt
                # overlaps the last DVE strip (4 strips measured net-zero:
                # they serialize at ~325ns each on the DVE)
                h = N_TILE // 2
                for c in range(2):
                    eng = nc.scalar if c % 2 == 0 else nc.sync
                    ob = outp.tile([P, h], mybir.dt.float16, tag="obs",
                                   name=f"ob_{j}_{m}_{c}")
                    nc.vector.tensor_add(
                        ob[:], psum[m][:, c * h:(c + 1) * h],
                        bias_rep[:, j * N_TILE + c * h:
                                 j * N_TILE + (c + 1) * h])
                    eng.dma_start(
                        out[m * P:(m + 1) * P,
                            j * N_TILE + c * h:j * N_TILE + (c + 1) * h],
                        ob[:])

            def do_group(j, m):
                psum[m] = psump.tile([P, N_TILE], mybir.dt.float32, tag="ps",
                                     name=f"ps_{j}_{m}")
                for kc in range(K_CHUNKS):
                    mm(j, m, kc)
                evict(j, m,
                      split=(j == N_TILES - 1 and m == M_TILES - 1))

            # ---- warm-up: column 0, m=0..6 k-chunk-outer so each arriving
            # w0/x chunk unlocks WARM_M matmuls (PE consumes a 352KB chunk
            # in 1.5us warm — slower than the exclusive sync queue delivers).
            # Dummy fill between the early chunks absorbs the cold-DMA wall
            # (~1MB by 14us) without letting the PE idle long enough to
            # re-gate the clock.
            FILL = {0: 16, 1: 6, 2: 4, 3: 2, 4: 2, 5: 1, 6: 1, 7: 1, 8: 2}
            for m in range(WARM_M):
                psum[m] = psump.tile([P, N_TILE], mybir.dt.float32, tag="ps",
                                     name=f"ps_0_{m}")
            for kc in range(K_CHUNKS):
                for m in range(WARM_M):
                    mm(0, m, kc)
                    if kc == 0 and m == 0:
                        dummies(FILL[0])
                dummies(FILL.get(kc + 1, 0))
            # bias_rep lands ~27us, warm-up ends ~38us: plain fused
            # evictions work (no decoupled copy needed)
            for m in range(WARM_M):
                evict(0, m)

            # ---- steady state: column-major, group-serial; w tiles were
            # all queued upfront, paced by the pool anti-deps ----
            for m in range(WARM_M, M_TILES):
                do_group(0, m)
            for j in range(1, N_TILES):
                for m in range(M_TILES):
                    do_group(j, m)

    nc.compile()
    _cached_nc = nc
    return nc


def make_in_maps(x, weight, bias):
    x = np.asarray(x)
    weight = np.asarray(weight)
    bias = np.ascontiguousarray(np.asarray(bias))
    assert x.dtype == np.float16 and weight.dtype == np.float16

    # quantize exactly as the reference does (RNE casts)
    x8 = x.astype(ml_dtypes.float8_e5m2)
    w8 = weight.astype(ml_dtypes.float8_e4m3fn)

    # weight [DOUT, DIN] -> [j, ki, ko, n]: wd[j,ki,ko,n] = w8[j*512+n,
    # ko*128+ki] (i.e. weight.T in per-tile K-major blocks)
    wd = np.ascontiguousarray(
        w8.reshape(N_TILES, N_TILE, K_SUB, P).transpose(0, 3, 2, 1))

    # bias pre-broadcast to all 128 partitions: one plain contiguous DMA
    brep = np.ascontiguousarray(np.broadcast_to(bias, (P, DOUT)))

    xf = x8.reshape(M_TOTAL, DIN)
    in_maps = []
    for c in range(NCORES):
        xc = xf[c * M_LOC:(c + 1) * M_LOC]
        # [M_LOC, DIN] -> [m-tile, ki, ko, m]: xd[t,ki,ko,m] = xc[t*128+m,
        # ko*128+ki]
        xdt = np.ascontiguousarray(
            xc.reshape(M_TILES, P, K_SUB, P).transpose(0, 3, 2, 1))
        # first WARM_M m-tiles also packed as one [ki, t, ko, m] block so
        # each ramp DMA delivers a k-slice for all warm groups at once
        xd01 = np.ascontiguousarray(xdt[:WARM_M].transpose(1, 0, 2, 3))
        in_maps.append({"xd01": xd01, "xd": np.ascontiguousarray(xdt[WARM_M:]),
                        "wd": wd, "brep": brep})
    return in_maps


def gather_out(results):
    out = np.concatenate([r["out"] for r in results], axis=0)
    return out.reshape(B, S, DOUT)


def kernel(x, weight, bias):
    from concourse.bass_utils import run_bass_kernel_spmd

    nc = _build()
    in_maps = make_in_maps(x, weight, bias)
    res = run_bass_kernel_spmd(nc, in_maps, core_ids=list(range(NCORES)))
    return gather_out(res.results)
